# revision 34
# baseline (speedup 1.0000x reference)
"""KANLinear fused kernel for 8x Trainium2 NeuronCores (fp16 + fp8 DoubleRow).

out[b,o] = silu(x) @ Wb^T + einsum('bik,oik->bo', bspline_basis(x), Ws)

Data-parallel over the 8192-token batch (1024 rows/core).

Contraction re-basis (exact): the 8-dim spline space on clipped z =
clip(x,-1,1) is spanned by {1, z, z^2, z^3, B2, B3, B4, B5} where Bg are
the four INNER cubic B-spline bumps (centers +-0.2, +-0.6). The four
poly rows ride fp16 matmuls (conditioning-insensitive); the four bump
rows ride fp8e4 DoubleRow matmuls (2 contraction rows per pass = 2x PE
throughput, measured 216ns/MM at N=512, same as fp16). Bump values are
exact local functions (partition-of-unity conditioning, kappa=1), so
e4m3 noise is not amplified. Per input-feature chunk: 6 matmul slots
(4 fp16 + 2 DoubleRow) instead of the 8 an all-fp16 kernel needs.

Accuracy stack (target ~1.5e-2 < 2e-2 gate):
  - bump features centered by mu=1/12, scaled -384: the clip point-mass
    values (B=0, 1/6, 2/3 at z=+-1) map to {+32, -32, -224}, all exactly
    representable in e4m3, so 31.7% of the inputs add zero feature noise.
  - host GPTQ: bump weights quantized with OBS compensation flowing into
    the EXACT fp16 poly rows + bias (H from an x subsample).
  - feature-side absorption: the deterministic e4m3 rounding error of
    each bump feature is projected onto the feature span and
    pre-subtracted from the weights.
  - product scale S=2^18 uniform across rows (alpha_r*beta_r=S), drains
    descale by 2^-18 into fp16 staging; host adds the f32 bias.

Bump evaluation on-chip with all cube scaling folded into
u' = |10z - 10c| (ACT Abs, input affine), batched as single [128, 4*BH]
ops per wave across the four bumps (4x fewer instructions, amortized
fixed overheads):
  an = min(u'-8, 0) = 10*a_neg, bn = min(an+4, 0) = 10*b_neg (DVE ts),
  qa = Square(an) (ACT), qb = bn*bn (DVE self-mult), ca = qa*an (DVE),
  cb = qb*bn (GPSIMD), cbs = 4*cb - 32 (ACT Copy),
  fb = ca - cbs = 1000*a_neg^3 - 4000*b_neg^3 + 32 = -384(B-1/12) -> fp8
  (two DVE ops so pair-0 DoubleRow matmuls start early). All PSUM drains
  on ACT (descale 2^-18 via Copy), keeping DVE, the pacing engine, lean.
"""
import sys
if "/opt/trn_rl_repo" not in sys.path:
    sys.path.insert(0, "/opt/trn_rl_repo")

import numpy as np
import ml_dtypes
import concourse.bass as bass
from concourse import bacc
import concourse.tile as tile
import concourse.mybir as mybir
from concourse.bass_utils import run_bass_kernel_spmd

AF = mybir.ActivationFunctionType
OP = mybir.AluOpType
DR = mybir.MatmulPerfMode.DoubleRow
F32, F16, F8 = mybir.dt.float32, mybir.dt.float16, mybir.dt.float8e4
E4NP = ml_dtypes.float8_e4m3  # TRN float8e4 (bias 7, max 240)

N_CORES = 8
B_FULL, I_FEAT, O_FEAT = 8192, 1024, 1024
B_LOC = B_FULL // N_CORES
BH = B_LOC // 2
N_CHUNK = I_FEAT // 128

MU = 1.0 / 12.0
S = 2.0 ** 18
ALPHA_B = -384.0
BETA_B = S / ALPHA_B            # -682.666…
CENTERS = (-0.6, -0.2, 0.2, 0.6)

_COMPILED = None


def _build_program():
    nc = bacc.Bacc("TRN2", target_bir_lowering=False, debug=False)
    xT = nc.dram_tensor("xT", [I_FEAT, B_LOC], F16, kind="ExternalInput").ap()
    wp = nc.dram_tensor("wp", [N_CHUNK, 128, 4, O_FEAT], F16,
                        kind="ExternalInput").ap()
    wb = nc.dram_tensor("wb", [N_CHUNK, 128, 4, O_FEAT], F8,
                        kind="ExternalInput").ap()
    out = nc.dram_tensor("out", [B_LOC, O_FEAT], F16, kind="ExternalOutput").ap()

    dve, act, gps = nc.vector, nc.scalar, nc.gpsimd

    # activation() resolves float bias via the const-AP registry; register
    # the Abs biases (-10*center) this kernel uses. No barrier needed: the
    # first consumer sits on the ACT queue behind ops that wait on DMAs.
    def reg_const(v):
        key = (F32, float(v))
        if key not in nc.const_aps.aps:
            t = nc.alloc_sbuf_tensor(f"constk-{len(nc.const_aps.aps)}", [128, 1], F32)
            nc.gpsimd.memset(t.ap(), float(v))
            nc.const_aps.aps[key] = t.ap()
    for c in CENTERS:
        reg_const(-10.0 * c)

    with tile.TileContext(nc) as tc:
        with tc.tile_pool(name="xin", bufs=2) as xpool, \
             tc.tile_pool(name="mid", bufs=2) as mid, \
             tc.tile_pool(name="feat", bufs=2) as fpool, \
             tc.tile_pool(name="wres", bufs=1) as wres, \
             tc.tile_pool(name="warm", bufs=1) as wpool, \
             tc.tile_pool(name="outsb", bufs=4) as opool, \
             tc.tile_pool(name="psum", bufs=1, space="PSUM") as pspool:

            # Resident weights: 8x fp16 poly (1MB) + 8x fp8 bump (0.5MB).
            # DMA ordering: batch-half-0 chunk-0 x FIRST (the feature chain
            # needs it within ~2us), then chunk-0 weights; later chunks'
            # weights prefetched from inside the chunk loop, staying ahead
            # of the matmul stream without blocking the x DMAs.
            wpoly_sb = [None] * N_CHUNK
            wbump_sb = [None] * N_CHUNK
            for f in range(N_CHUNK):
                wpoly_sb[f] = wres.tile([128, 4, O_FEAT], F16,
                                        tag=f"wp{f}", name=f"wp{f}")
                wbump_sb[f] = wres.tile([128, 4, O_FEAT], F8,
                                        tag=f"wb{f}", name=f"wb{f}")
            # Weight prefetches ride the GPSIMD-triggered DMA queue so
            # the x chunk DMAs (sync queue) never wait FIFO behind 1.5MB
            # weight transfers.
            xin0 = xpool.tile([128, BH], F16, tag="x", name="x")
            nc.sync.dma_start(xin0[:], xT[0:128, 0:BH])
            nc.gpsimd.dma_start(wpoly_sb[0][:], wp[0, :, :, :])
            nc.gpsimd.dma_start(wbump_sb[0][:], wb[0, :, :, :])

            # HAM warmup: dummy matmuls so the PE clock-gate reaches 8/8
            # before the real stream begins.
            warm16 = wpool.tile([128, 512], F16, tag="wrm", name="wrm")
            nc.gpsimd.memset(warm16[:], 0.0)
            warm_ps = pspool.tile([128, 512], F32, tag="ps0", name="ps0w")
            for _ in range(12):
                nc.tensor.matmul(warm_ps[:], warm16[:, 0:128], warm16[:],
                                 start=True, stop=True)

            for bh in range(2):
                if bh == 1:
                    # bridge the inter-half PE gap so the HAM clock-gate
                    # stays at 8/8 through the batch-half transition
                    for _ in range(12):
                        nc.tensor.matmul(warm_ps[:], warm16[:, 0:128],
                                         warm16[:], start=True, stop=True)
                psums = [pspool.tile([128, 512], F32, tag=f"ps{j}", name=f"ps{j}")
                         for j in range(8)]   # j = bt*2 + oh
                tail_feats = None
                for f in range(N_CHUNK):
                    if bh == 0 and f == 0:
                        xin = xin0
                    else:
                        xin = xpool.tile([128, BH], F16, tag="x", name="x")
                        nc.sync.dma_start(
                            xin[:],
                            xT[f * 128:(f + 1) * 128, bh * BH:(bh + 1) * BH])
                    if bh == 0 and f + 1 < N_CHUNK:
                        nc.gpsimd.dma_start(wpoly_sb[f + 1][:], wp[f + 1, :, :, :])
                        nc.gpsimd.dma_start(wbump_sb[f + 1][:], wb[f + 1, :, :, :])

                    # fp16 rows: silu, z, z^2, z^3 (alpha=1; scales live in
                    # the fp16 weights)
                    sl = fpool.tile([128, BH], F16, tag="silu", name="silu")
                    act.activation(sl[:], xin[:], AF.Silu)
                    zc = fpool.tile([128, BH], F16, tag="zc", name="zc")
                    dve.tensor_scalar(zc[:], xin[:], -1.0, 1.0, OP.max, OP.min)
                    z2 = fpool.tile([128, BH], F16, tag="z2", name="z2")
                    gps.tensor_tensor(z2[:], zc[:], zc[:], OP.mult)
                    z3 = fpool.tile([128, BH], F16, tag="z3", name="z3")
                    dve.tensor_tensor(z3[:], z2[:], zc[:], OP.mult)
                    prows = (sl, zc, z2, z3)

                    # fp8 bump rows -> one tile [128, 4, BH].
                    # u' = 10|zc-c| folds all cube scaling: an' = 10*an,
                    # an'^3 = 1000*an^3, so plain products give the scaled
                    # cubes. fb = an'^3 - (4*bn'^3 - 32) = -384(B - 1/12).
                    # All four bumps are processed as single batched
                    # [128, 4*BH] ops per wave (4x fewer instructions,
                    # amortized fixed overheads and semaphores).
                    fb = fpool.tile([128, 4, BH], F8, tag="fb", name="fb")
                    ua = mid.tile([128, 4, BH], F16, tag="ua", name="ua")
                    for j, c in enumerate(CENTERS):
                        act.activation(ua[:, j, :], zc[:], AF.Abs, scale=10.0,
                                       bias=float(-10.0 * c))
                    ana = mid.tile([128, 4, BH], F16, tag="ana", name="ana")
                    dve.tensor_scalar(ana[:], ua[:], 8.0, 0.0,
                                      OP.subtract, OP.min)
                    bna = mid.tile([128, 4, BH], F16, tag="bna", name="bna")
                    dve.tensor_scalar(bna[:], ana[:], 4.0, 0.0,
                                      OP.add, OP.min)
                    qaa = mid.tile([128, 4, BH], F16, tag="qaa", name="qaa")
                    act.activation(qaa[:], ana[:], AF.Square)
                    qba = mid.tile([128, 4, BH], F16, tag="qba", name="qba")
                    dve.tensor_tensor(qba[:], bna[:], bna[:], OP.mult)
                    caa = mid.tile([128, 4, BH], F16, tag="caa", name="caa")
                    dve.tensor_tensor(caa[:], qaa[:], ana[:], OP.mult)
                    cba = mid.tile([128, 4, BH], F16, tag="cba", name="cba")
                    gps.tensor_tensor(cba[:], qba[:], bna[:], OP.mult)
                    cbsa = mid.tile([128, 4, BH], F16, tag="cbsa", name="cbsa")
                    act.activation(cbsa[:], cba[:], AF.Copy,
                                   scale=4.0, bias=-32.0)
                    dve.tensor_tensor(fb[:, 0:2, :], caa[:, 0:2, :],
                                      cbsa[:, 0:2, :], OP.subtract)
                    dve.tensor_tensor(fb[:, 2:4, :], caa[:, 2:4, :],
                                      cbsa[:, 2:4, :], OP.subtract)

                    # matmuls: 6 slots per chunk (4 fp16 + 2 DoubleRow),
                    # features stationary, one lhsT feeds both oh halves.
                    # Final chunk handled bank-major below for drain overlap.
                    if f == N_CHUNK - 1:
                        tail_feats = (prows, fb)
                        continue
                    for bt in range(4):
                        for r in range(4):
                            lhsT = prows[r][:, bt * 128:(bt + 1) * 128]
                            for oh in range(2):
                                nc.tensor.matmul(
                                    psums[bt * 2 + oh][:], lhsT,
                                    wpoly_sb[f][:, r, oh * 512:(oh + 1) * 512],
                                    start=(f == 0 and r == 0), stop=False)
                        for p in range(2):
                            lhsT = fb[:, 2 * p:2 * p + 2, bt * 128:(bt + 1) * 128]
                            for oh in range(2):
                                nc.tensor.matmul(
                                    psums[bt * 2 + oh][:], lhsT,
                                    wb_pair(wbump_sb[f], p, oh),
                                    start=False, stop=False, perf_mode=DR)

                # bank-major tail over the last chunk: bank j finishes its 6
                # contributions then drains while the PE works on later banks.
                prows, fb = tail_feats
                fl = N_CHUNK - 1
                for j in range(8):
                    bt, oh = j // 2, j % 2
                    for r in range(4):
                        nc.tensor.matmul(
                            psums[j][:],
                            prows[r][:, bt * 128:(bt + 1) * 128],
                            wpoly_sb[fl][:, r, oh * 512:(oh + 1) * 512],
                            start=False, stop=False)
                    for p in range(2):
                        nc.tensor.matmul(
                            psums[j][:],
                            fb[:, 2 * p:2 * p + 2, bt * 128:(bt + 1) * 128],
                            wb_pair(wbump_sb[fl], p, oh),
                            start=False, stop=(p == 1), perf_mode=DR)
                    rows = slice(bh * BH + bt * 128, bh * BH + (bt + 1) * 128)
                    cols = slice(oh * 512, (oh + 1) * 512)
                    ob = opool.tile([128, 512], F16, tag=f"obt{oh}",
                                    name=f"obt{oh}")
                    # mid-kernel drains stay off DVE (the pacing engine);
                    # in the final tail DVE is idle, so alternate engines
                    # to halve the post-stream drain chain.
                    if bh == 1 and oh == 0:
                        dve.tensor_scalar_mul(ob[:], psums[j][:], 1.0 / S)
                    else:
                        act.activation(ob[:], psums[j][:], AF.Copy,
                                       scale=1.0 / S)
                    nc.sync.dma_start(out[rows, cols], ob[:])
    nc.compile()
    return nc


def wb_pair(wtile, p, oh):
    """rhs AP [128, 2, 512] for DoubleRow pair p, output half oh."""
    return wtile[:, 2 * p:2 * p + 2, oh * 512:(oh + 1) * 512]


def _get_program():
    global _COMPILED
    if _COMPILED is None:
        _COMPILED = _build_program()
    return _COMPILED


# ---------------- host-side weight preparation ----------------

_GRID = np.linspace(-2.2, 2.2, 12)
_H_KNOT = 0.4


def _bspline_basis_np(z):
    xg = z[..., None]
    basis = ((xg >= _GRID[:-1]) & (xg < _GRID[1:])).astype(np.float64)
    for k in range(1, 4):
        ld = _GRID[k:-1] - _GRID[:-(k + 1)]; ld = np.where(ld == 0, 1, ld)
        rd = _GRID[k + 1:] - _GRID[1:-k]; rd = np.where(rd == 0, 1, rd)
        basis = ((xg - _GRID[:-(k + 1)]) / ld * basis[..., :-1]
                 + (_GRID[k + 1:] - xg) / rd * basis[..., 1:])
    return basis


def _bump_np(z, c):
    u = np.abs(z - c) / _H_KNOT
    return (np.maximum(2 - u, 0) ** 3 - 4 * np.maximum(1 - u, 0) ** 3) / 6


def _features_np(z):
    return np.stack([np.ones_like(z), z, z * z, z ** 3,
                     _bump_np(z, -0.6), _bump_np(z, -0.2),
                     _bump_np(z, 0.2), _bump_np(z, 0.6)], axis=-1)


def _fold_matrix():
    zs = np.linspace(-1, 1, 20001)
    M, *_ = np.linalg.lstsq(_features_np(zs), _bspline_basis_np(zs), rcond=None)
    return M   # (8 features, 8 basis)


_M_FOLD = None


def _q8(v):
    return np.clip(v, -224, 224).astype(E4NP).astype(np.float64)


def _prep_weights(x, base_weight, spline_weight):
    global _M_FOLD
    if _M_FOLD is None:
        _M_FOLD = _fold_matrix()
    bw = np.ascontiguousarray(base_weight, dtype=np.float64)
    sw = np.ascontiguousarray(spline_weight, dtype=np.float64)
    O, I = bw.shape

    W = np.einsum('oig,fg->oif', sw, _M_FOLD)       # (O, I, 8)
    W[:, :, 0] += MU * W[:, :, 4:8].sum(axis=2)     # centering -> bias col

    # feature stats on an x subsample (centered features, device scaling)
    rng = np.random.default_rng(12345)
    zs = np.clip(x[rng.choice(x.shape[0], 512, replace=False)].ravel(), -1, 1)
    Fs = _features_np(zs)
    Fs[:, 4:8] -= MU
    H = Fs.T @ Fs / len(zs)

    # feature-side absorption: project deterministic e4m3 feature error of
    # each bump row onto the feature span; pre-subtract from weights.
    dF = np.empty_like(Fs[:, 4:8])
    for j in range(4):
        v = Fs[:, 4 + j] * ALPHA_B
        dF[:, j] = _q8(v) / ALPHA_B - Fs[:, 4 + j]
    Hreg = H + 1e-10 * np.eye(8) * H.max()
    C = np.linalg.solve(Hreg, Fs.T @ dF / len(zs))   # (8, 4) projection coeffs
    W -= np.einsum('oig,fg->oif', W[:, :, 4:8], C)

    # GPTQ: quantize bump rows (e4m3 at BETA_B) with OBS compensation into
    # all not-yet-quantized coords (incl. exact poly + bias rows).
    Wq = W.reshape(-1, 8)
    Hi = np.linalg.inv(Hreg)
    done = []
    for g in (4, 5, 6, 7):
        qg = _q8(Wq[:, g] * BETA_B) / BETA_B
        e = (Wq[:, g] - qg) / Hi[g, g]
        Wq[:, g] = qg
        done.append(g)
        rem = [j for j in range(8) if j not in done]
        Wq[:, rem] -= np.outer(e, Hi[g, rem])
        Hi = Hi - np.outer(Hi[:, g], Hi[g, :]) / Hi[g, g]
        Hi[g, :] = 0; Hi[:, g] = 0; Hi[g, g] = 1.0
    W = Wq.reshape(O, I, 8)

    bias = W[:, :, 0].sum(axis=1)                   # (O,)

    # device arrays
    wp = np.empty((N_CHUNK, 128, 4, O_FEAT), dtype=np.float16)
    wb8 = np.empty((N_CHUNK, 128, 4, O_FEAT), dtype=E4NP)
    for f in range(N_CHUNK):
        rows = slice(f * 128, (f + 1) * 128)
        wp[f, :, 0, :] = (bw.T[rows, :] * S).astype(np.float16)
        for r in (1, 2, 3):
            wp[f, :, r, :] = (W[:, rows, r].T * S).astype(np.float16)
        for j in range(4):
            wb8[f, :, j, :] = np.clip(W[:, rows, 4 + j].T * BETA_B,
                                      -224, 224).astype(E4NP)
    return wp, wb8, bias.astype(np.float32)


def _run(x, base_weight, spline_weight, trace=False, tmpdir=None):
    nc = _get_program()
    x64 = np.ascontiguousarray(x, dtype=np.float64)
    x16 = x64.astype(np.float16)
    wp, wb8, bias = _prep_weights(x64, base_weight, spline_weight)
    in_maps = []
    for c in range(N_CORES):
        xc = np.ascontiguousarray(x16[c * B_LOC:(c + 1) * B_LOC, :].T)
        in_maps.append({"xT": xc, "wp": wp, "wb": wb8})
    res = run_bass_kernel_spmd(nc, in_maps, core_ids=list(range(N_CORES)),
                               trace=trace, tmpdir=tmpdir)
    full = np.concatenate([res.results[c]["out"] for c in range(N_CORES)],
                          axis=0).astype(np.float32)
    full += bias[None, :]
    return full, res


def kernel(x, base_weight, spline_weight):
    out, _ = _run(x, base_weight, spline_weight, trace=False)
    return out


# revision 35
# speedup vs baseline: 1.0792x; 1.0792x over previous
"""KANLinear fused kernel for 8x Trainium2 NeuronCores (fp16 + fp8 DoubleRow).

out[b,o] = silu(x) @ Wb^T + einsum('bik,oik->bo', bspline_basis(x), Ws)

Data-parallel over the 8192-token batch (1024 rows/core).

Contraction re-basis (exact): the 8-dim spline space on clipped z =
clip(x,-1,1) is spanned by {1, z, z^2, z^3, B2, B3, B4, B5} where Bg are
the four INNER cubic B-spline bumps (centers +-0.2, +-0.6). The four
poly rows ride fp16 matmuls (conditioning-insensitive); the four bump
rows ride fp8e4 DoubleRow matmuls (2 contraction rows per pass = 2x PE
throughput, measured 216ns/MM at N=512, same as fp16). Bump values are
exact local functions (partition-of-unity conditioning, kappa=1), so
e4m3 noise is not amplified. Per input-feature chunk: 6 matmul slots
(4 fp16 + 2 DoubleRow) instead of the 8 an all-fp16 kernel needs.

Accuracy stack (target ~1.5e-2 < 2e-2 gate):
  - bump features centered by mu=1/12, scaled -384: the clip point-mass
    values (B=0, 1/6, 2/3 at z=+-1) map to {+32, -32, -224}, all exactly
    representable in e4m3, so 31.7% of the inputs add zero feature noise.
  - host GPTQ: bump weights quantized with OBS compensation flowing into
    the EXACT fp16 poly rows + bias (H from an x subsample).
  - feature-side absorption: the deterministic e4m3 rounding error of
    each bump feature is projected onto the feature span and
    pre-subtracted from the weights.
  - product scale S=2^18 uniform across rows (alpha_r*beta_r=S), drains
    descale by 2^-18 into fp16 staging; host adds the f32 bias.

Bump evaluation on-chip with all cube scaling folded into
u' = |10z - 10c| (ACT Abs, input affine), batched as single [128, 4*BH]
ops per wave across the four bumps (4x fewer instructions, amortized
fixed overheads):
  an = min(u'-8, 0) = 10*a_neg, bn = min(an+4, 0) = 10*b_neg (DVE ts),
  qa = Square(an) (ACT), qb = bn*bn (DVE self-mult), ca = qa*an (DVE),
  cb = qb*bn (GPSIMD), cbs = 4*cb - 32 (ACT Copy),
  fb = ca - cbs = 1000*a_neg^3 - 4000*b_neg^3 + 32 = -384(B-1/12) -> fp8
  (two DVE ops so pair-0 DoubleRow matmuls start early). All PSUM drains
  on ACT (descale 2^-18 via Copy), keeping DVE, the pacing engine, lean.
"""
import sys
if "/opt/trn_rl_repo" not in sys.path:
    sys.path.insert(0, "/opt/trn_rl_repo")

import numpy as np
import ml_dtypes
import concourse.bass as bass
from concourse import bacc
import concourse.tile as tile
import concourse.mybir as mybir
from concourse.bass_utils import run_bass_kernel_spmd

AF = mybir.ActivationFunctionType
OP = mybir.AluOpType
DR = mybir.MatmulPerfMode.DoubleRow
F32, F16, F8 = mybir.dt.float32, mybir.dt.float16, mybir.dt.float8e4
E4NP = ml_dtypes.float8_e4m3  # TRN float8e4 (bias 7, max 240)

N_CORES = 8
B_FULL, I_FEAT, O_FEAT = 8192, 1024, 1024
B_LOC = B_FULL // N_CORES
BH = B_LOC // 2
N_CHUNK = I_FEAT // 128

MU = 1.0 / 12.0
S = 2.0 ** 18
ALPHA_B = -384.0
BETA_B = S / ALPHA_B            # -682.666…
CENTERS = (-0.6, -0.2, 0.2, 0.6)

_COMPILED = None


def _build_program():
    nc = bacc.Bacc("TRN2", target_bir_lowering=False, debug=False)
    xT = nc.dram_tensor("xT", [I_FEAT, B_LOC], F16, kind="ExternalInput").ap()
    wp = nc.dram_tensor("wp", [N_CHUNK, 128, 4, O_FEAT], F16,
                        kind="ExternalInput").ap()
    wb = nc.dram_tensor("wb", [N_CHUNK, 128, 4, O_FEAT], F8,
                        kind="ExternalInput").ap()
    out = nc.dram_tensor("out", [B_LOC, O_FEAT], F16, kind="ExternalOutput").ap()

    dve, act, gps = nc.vector, nc.scalar, nc.gpsimd

    # activation() resolves float bias via the const-AP registry; register
    # the Abs biases (-10*center) this kernel uses. No barrier needed: the
    # first consumer sits on the ACT queue behind ops that wait on DMAs.
    def reg_const(v):
        key = (F32, float(v))
        if key not in nc.const_aps.aps:
            t = nc.alloc_sbuf_tensor(f"constk-{len(nc.const_aps.aps)}", [128, 1], F32)
            nc.gpsimd.memset(t.ap(), float(v))
            nc.const_aps.aps[key] = t.ap()
    for c in CENTERS:
        reg_const(-10.0 * c)

    with tile.TileContext(nc) as tc:
        with tc.tile_pool(name="xin", bufs=2) as xpool, \
             tc.tile_pool(name="mid", bufs=2) as mid, \
             tc.tile_pool(name="feat", bufs=2) as fpool, \
             tc.tile_pool(name="wres", bufs=1) as wres, \
             tc.tile_pool(name="warm", bufs=1) as wpool, \
             tc.tile_pool(name="outsb", bufs=4) as opool, \
             tc.tile_pool(name="psum", bufs=1, space="PSUM") as pspool:

            # Resident weights: 8x fp16 poly (1MB) + 8x fp8 bump (0.5MB).
            # DMA ordering: batch-half-0 chunk-0 x FIRST (the feature chain
            # needs it within ~2us), then chunk-0 weights; later chunks'
            # weights prefetched from inside the chunk loop, staying ahead
            # of the matmul stream without blocking the x DMAs.
            wpoly_sb = [None] * N_CHUNK
            wbump_sb = [None] * N_CHUNK
            for f in range(N_CHUNK):
                wpoly_sb[f] = wres.tile([128, 4, O_FEAT], F16,
                                        tag=f"wp{f}", name=f"wp{f}")
                wbump_sb[f] = wres.tile([128, 4, O_FEAT], F8,
                                        tag=f"wb{f}", name=f"wb{f}")
            # x first on the queue; weight DMAs follow (measured faster
            # than triggering weights from the ACT or GPSIMD queues, whose
            # compute ops delay the triggers).
            xin0 = xpool.tile([128, BH], F16, tag="x", name="x")
            nc.sync.dma_start(xin0[:], xT[0:128, 0:BH])
            nc.sync.dma_start(wpoly_sb[0][:], wp[0, :, :, :])
            nc.sync.dma_start(wbump_sb[0][:], wb[0, :, :, :])

            # HAM warmup: dummy matmuls so the PE clock-gate reaches 8/8
            # before the real stream begins.
            warm16 = wpool.tile([128, 512], F16, tag="wrm", name="wrm")
            nc.gpsimd.memset(warm16[:], 0.0)
            warm_ps = pspool.tile([128, 512], F32, tag="ps0", name="ps0w")
            for _ in range(12):
                nc.tensor.matmul(warm_ps[:], warm16[:, 0:128], warm16[:],
                                 start=True, stop=True)

            for bh in range(2):
                if bh == 1:
                    # bridge the inter-half PE gap so the HAM clock-gate
                    # stays at 8/8 through the batch-half transition
                    for _ in range(12):
                        nc.tensor.matmul(warm_ps[:], warm16[:, 0:128],
                                         warm16[:], start=True, stop=True)
                psums = [pspool.tile([128, 512], F32, tag=f"ps{j}", name=f"ps{j}")
                         for j in range(8)]   # j = bt*2 + oh
                tail_feats = None
                for f in range(N_CHUNK):
                    if bh == 0 and f == 0:
                        xin = xin0
                    else:
                        xin = xpool.tile([128, BH], F16, tag="x", name="x")
                        nc.sync.dma_start(
                            xin[:],
                            xT[f * 128:(f + 1) * 128, bh * BH:(bh + 1) * BH])
                    if bh == 0 and f + 1 < N_CHUNK:
                        nc.sync.dma_start(wpoly_sb[f + 1][:], wp[f + 1, :, :, :])
                        nc.sync.dma_start(wbump_sb[f + 1][:], wb[f + 1, :, :, :])

                    # fp16 rows: silu, z, z^2, z^3 (alpha=1; scales live in
                    # the fp16 weights)
                    sl = fpool.tile([128, BH], F16, tag="silu", name="silu")
                    act.activation(sl[:], xin[:], AF.Silu)
                    zc = fpool.tile([128, BH], F16, tag="zc", name="zc")
                    dve.tensor_scalar(zc[:], xin[:], -1.0, 1.0, OP.max, OP.min)
                    z2 = fpool.tile([128, BH], F16, tag="z2", name="z2")
                    gps.tensor_tensor(z2[:], zc[:], zc[:], OP.mult)
                    z3 = fpool.tile([128, BH], F16, tag="z3", name="z3")
                    dve.tensor_tensor(z3[:], z2[:], zc[:], OP.mult)
                    prows = (sl, zc, z2, z3)

                    # fp8 bump rows -> one tile [128, 4, BH].
                    # u' = 10|zc-c| folds all cube scaling: an' = 10*an,
                    # an'^3 = 1000*an^3, so plain products give the scaled
                    # cubes. fb = an'^3 - (4*bn'^3 - 32) = -384(B - 1/12).
                    # All four bumps are processed as single batched
                    # [128, 4*BH] ops per wave (4x fewer instructions,
                    # amortized fixed overheads and semaphores).
                    fb = fpool.tile([128, 4, BH], F8, tag="fb", name="fb")
                    ua = mid.tile([128, 4, BH], F16, tag="ua", name="ua")
                    for j, c in enumerate(CENTERS):
                        act.activation(ua[:, j, :], zc[:], AF.Abs, scale=10.0,
                                       bias=float(-10.0 * c))
                    ana = mid.tile([128, 4, BH], F16, tag="ana", name="ana")
                    dve.tensor_scalar(ana[:], ua[:], 8.0, 0.0,
                                      OP.subtract, OP.min)
                    bna = mid.tile([128, 4, BH], F16, tag="bna", name="bna")
                    dve.tensor_scalar(bna[:], ana[:], 4.0, 0.0,
                                      OP.add, OP.min)
                    qaa = mid.tile([128, 4, BH], F16, tag="qaa", name="qaa")
                    act.activation(qaa[:], ana[:], AF.Square)
                    qba = mid.tile([128, 4, BH], F16, tag="qba", name="qba")
                    dve.tensor_tensor(qba[:], bna[:], bna[:], OP.mult)
                    caa = mid.tile([128, 4, BH], F16, tag="caa", name="caa")
                    dve.tensor_tensor(caa[:], qaa[:], ana[:], OP.mult)
                    cba = mid.tile([128, 4, BH], F16, tag="cba", name="cba")
                    gps.tensor_tensor(cba[:], qba[:], bna[:], OP.mult)
                    cbsa = mid.tile([128, 4, BH], F16, tag="cbsa", name="cbsa")
                    act.activation(cbsa[:], cba[:], AF.Copy,
                                   scale=4.0, bias=-32.0)
                    dve.tensor_tensor(fb[:, 0:2, :], caa[:, 0:2, :],
                                      cbsa[:, 0:2, :], OP.subtract)
                    dve.tensor_tensor(fb[:, 2:4, :], caa[:, 2:4, :],
                                      cbsa[:, 2:4, :], OP.subtract)

                    # matmuls: 6 slots per chunk (4 fp16 + 2 DoubleRow),
                    # features stationary, one lhsT feeds both oh halves.
                    # Final chunk handled bank-major below for drain overlap.
                    if f == N_CHUNK - 1:
                        tail_feats = (prows, fb)
                        continue
                    for bt in range(4):
                        for r in range(4):
                            lhsT = prows[r][:, bt * 128:(bt + 1) * 128]
                            for oh in range(2):
                                nc.tensor.matmul(
                                    psums[bt * 2 + oh][:], lhsT,
                                    wpoly_sb[f][:, r, oh * 512:(oh + 1) * 512],
                                    start=(f == 0 and r == 0), stop=False)
                        for p in range(2):
                            lhsT = fb[:, 2 * p:2 * p + 2, bt * 128:(bt + 1) * 128]
                            for oh in range(2):
                                nc.tensor.matmul(
                                    psums[bt * 2 + oh][:], lhsT,
                                    wb_pair(wbump_sb[f], p, oh),
                                    start=False, stop=False, perf_mode=DR)

                # bank-major tail over the last chunk: bank j finishes its 6
                # contributions then drains while the PE works on later banks.
                prows, fb = tail_feats
                fl = N_CHUNK - 1
                for j in range(8):
                    bt, oh = j // 2, j % 2
                    for r in range(4):
                        nc.tensor.matmul(
                            psums[j][:],
                            prows[r][:, bt * 128:(bt + 1) * 128],
                            wpoly_sb[fl][:, r, oh * 512:(oh + 1) * 512],
                            start=False, stop=False)
                    for p in range(2):
                        nc.tensor.matmul(
                            psums[j][:],
                            fb[:, 2 * p:2 * p + 2, bt * 128:(bt + 1) * 128],
                            wb_pair(wbump_sb[fl], p, oh),
                            start=False, stop=(p == 1), perf_mode=DR)
                    rows = slice(bh * BH + bt * 128, bh * BH + (bt + 1) * 128)
                    cols = slice(oh * 512, (oh + 1) * 512)
                    ob = opool.tile([128, 512], F16, tag=f"obt{oh}",
                                    name=f"obt{oh}")
                    # mid-kernel drains stay off DVE (the pacing engine);
                    # in the final tail DVE is idle, so alternate engines
                    # to halve the post-stream drain chain.
                    if bh == 1 and oh == 0:
                        dve.tensor_scalar_mul(ob[:], psums[j][:], 1.0 / S)
                    else:
                        act.activation(ob[:], psums[j][:], AF.Copy,
                                       scale=1.0 / S)
                    nc.sync.dma_start(out[rows, cols], ob[:])
    nc.compile()
    return nc


def wb_pair(wtile, p, oh):
    """rhs AP [128, 2, 512] for DoubleRow pair p, output half oh."""
    return wtile[:, 2 * p:2 * p + 2, oh * 512:(oh + 1) * 512]


def _get_program():
    global _COMPILED
    if _COMPILED is None:
        _COMPILED = _build_program()
    return _COMPILED


# ---------------- host-side weight preparation ----------------

_GRID = np.linspace(-2.2, 2.2, 12)
_H_KNOT = 0.4


def _bspline_basis_np(z):
    xg = z[..., None]
    basis = ((xg >= _GRID[:-1]) & (xg < _GRID[1:])).astype(np.float64)
    for k in range(1, 4):
        ld = _GRID[k:-1] - _GRID[:-(k + 1)]; ld = np.where(ld == 0, 1, ld)
        rd = _GRID[k + 1:] - _GRID[1:-k]; rd = np.where(rd == 0, 1, rd)
        basis = ((xg - _GRID[:-(k + 1)]) / ld * basis[..., :-1]
                 + (_GRID[k + 1:] - xg) / rd * basis[..., 1:])
    return basis


def _bump_np(z, c):
    u = np.abs(z - c) / _H_KNOT
    return (np.maximum(2 - u, 0) ** 3 - 4 * np.maximum(1 - u, 0) ** 3) / 6


def _features_np(z):
    return np.stack([np.ones_like(z), z, z * z, z ** 3,
                     _bump_np(z, -0.6), _bump_np(z, -0.2),
                     _bump_np(z, 0.2), _bump_np(z, 0.6)], axis=-1)


def _fold_matrix():
    zs = np.linspace(-1, 1, 20001)
    M, *_ = np.linalg.lstsq(_features_np(zs), _bspline_basis_np(zs), rcond=None)
    return M   # (8 features, 8 basis)


_M_FOLD = None


def _q8(v):
    return np.clip(v, -224, 224).astype(E4NP).astype(np.float64)


def _prep_weights(x, base_weight, spline_weight):
    global _M_FOLD
    if _M_FOLD is None:
        _M_FOLD = _fold_matrix()
    bw = np.ascontiguousarray(base_weight, dtype=np.float64)
    sw = np.ascontiguousarray(spline_weight, dtype=np.float64)
    O, I = bw.shape

    W = np.einsum('oig,fg->oif', sw, _M_FOLD)       # (O, I, 8)
    W[:, :, 0] += MU * W[:, :, 4:8].sum(axis=2)     # centering -> bias col

    # feature stats on an x subsample (centered features, device scaling)
    rng = np.random.default_rng(12345)
    zs = np.clip(x[rng.choice(x.shape[0], 512, replace=False)].ravel(), -1, 1)
    Fs = _features_np(zs)
    Fs[:, 4:8] -= MU
    H = Fs.T @ Fs / len(zs)

    # feature-side absorption: project deterministic e4m3 feature error of
    # each bump row onto the feature span; pre-subtract from weights.
    dF = np.empty_like(Fs[:, 4:8])
    for j in range(4):
        v = Fs[:, 4 + j] * ALPHA_B
        dF[:, j] = _q8(v) / ALPHA_B - Fs[:, 4 + j]
    Hreg = H + 1e-10 * np.eye(8) * H.max()
    C = np.linalg.solve(Hreg, Fs.T @ dF / len(zs))   # (8, 4) projection coeffs
    W -= np.einsum('oig,fg->oif', W[:, :, 4:8], C)

    # GPTQ: quantize bump rows (e4m3 at BETA_B) with OBS compensation into
    # all not-yet-quantized coords (incl. exact poly + bias rows).
    Wq = W.reshape(-1, 8)
    Hi = np.linalg.inv(Hreg)
    done = []
    for g in (4, 5, 6, 7):
        qg = _q8(Wq[:, g] * BETA_B) / BETA_B
        e = (Wq[:, g] - qg) / Hi[g, g]
        Wq[:, g] = qg
        done.append(g)
        rem = [j for j in range(8) if j not in done]
        Wq[:, rem] -= np.outer(e, Hi[g, rem])
        Hi = Hi - np.outer(Hi[:, g], Hi[g, :]) / Hi[g, g]
        Hi[g, :] = 0; Hi[:, g] = 0; Hi[g, g] = 1.0
    W = Wq.reshape(O, I, 8)

    bias = W[:, :, 0].sum(axis=1)                   # (O,)

    # device arrays
    wp = np.empty((N_CHUNK, 128, 4, O_FEAT), dtype=np.float16)
    wb8 = np.empty((N_CHUNK, 128, 4, O_FEAT), dtype=E4NP)
    for f in range(N_CHUNK):
        rows = slice(f * 128, (f + 1) * 128)
        wp[f, :, 0, :] = (bw.T[rows, :] * S).astype(np.float16)
        for r in (1, 2, 3):
            wp[f, :, r, :] = (W[:, rows, r].T * S).astype(np.float16)
        for j in range(4):
            wb8[f, :, j, :] = np.clip(W[:, rows, 4 + j].T * BETA_B,
                                      -224, 224).astype(E4NP)
    return wp, wb8, bias.astype(np.float32)


def _run(x, base_weight, spline_weight, trace=False, tmpdir=None):
    nc = _get_program()
    x64 = np.ascontiguousarray(x, dtype=np.float64)
    x16 = x64.astype(np.float16)
    wp, wb8, bias = _prep_weights(x64, base_weight, spline_weight)
    in_maps = []
    for c in range(N_CORES):
        xc = np.ascontiguousarray(x16[c * B_LOC:(c + 1) * B_LOC, :].T)
        in_maps.append({"xT": xc, "wp": wp, "wb": wb8})
    res = run_bass_kernel_spmd(nc, in_maps, core_ids=list(range(N_CORES)),
                               trace=trace, tmpdir=tmpdir)
    full = np.concatenate([res.results[c]["out"] for c in range(N_CORES)],
                          axis=0).astype(np.float32)
    full += bias[None, :]
    return full, res


def kernel(x, base_weight, spline_weight):
    out, _ = _run(x, base_weight, spline_weight, trace=False)
    return out


# revision 36
# speedup vs baseline: 1.0802x; 1.0009x over previous
"""KANLinear fused kernel for 8x Trainium2 NeuronCores (fp16 + fp8 DoubleRow).

out[b,o] = silu(x) @ Wb^T + einsum('bik,oik->bo', bspline_basis(x), Ws)

Data-parallel over the 8192-token batch (1024 rows/core).

Contraction re-basis (exact): the 8-dim spline space on clipped z =
clip(x,-1,1) is spanned by {1, z, z^2, z^3, B2, B3, B4, B5} where Bg are
the four INNER cubic B-spline bumps (centers +-0.2, +-0.6). The four
poly rows ride fp16 matmuls (conditioning-insensitive); the four bump
rows ride fp8e4 DoubleRow matmuls (2 contraction rows per pass = 2x PE
throughput, measured 216ns/MM at N=512, same as fp16). Bump values are
exact local functions (partition-of-unity conditioning, kappa=1), so
e4m3 noise is not amplified. Per input-feature chunk: 6 matmul slots
(4 fp16 + 2 DoubleRow) instead of the 8 an all-fp16 kernel needs.

Accuracy stack (target ~1.5e-2 < 2e-2 gate):
  - bump features centered by mu=1/12, scaled -384: the clip point-mass
    values (B=0, 1/6, 2/3 at z=+-1) map to {+32, -32, -224}, all exactly
    representable in e4m3, so 31.7% of the inputs add zero feature noise.
  - host GPTQ: bump weights quantized with OBS compensation flowing into
    the EXACT fp16 poly rows + bias (H from an x subsample).
  - feature-side absorption: the deterministic e4m3 rounding error of
    each bump feature is projected onto the feature span and
    pre-subtracted from the weights.
  - product scale S=2^18 uniform across rows (alpha_r*beta_r=S), drains
    descale by 2^-18 into fp16 staging; host adds the f32 bias.

Bump evaluation on-chip with all cube scaling folded into
u' = |10z - 10c| (ACT Abs, input affine), batched as single [128, 4*BH]
ops per wave across the four bumps (4x fewer instructions, amortized
fixed overheads):
  an = min(u'-8, 0) = 10*a_neg, bn = min(an+4, 0) = 10*b_neg (DVE ts),
  qa = Square(an) (ACT), qb = bn*bn (DVE self-mult), ca = qa*an (DVE),
  cb = qb*bn (GPSIMD), cbs = 4*cb - 32 (ACT Copy),
  fb = ca - cbs = 1000*a_neg^3 - 4000*b_neg^3 + 32 = -384(B-1/12) -> fp8
  (two DVE ops so pair-0 DoubleRow matmuls start early). All PSUM drains
  on ACT (descale 2^-18 via Copy), keeping DVE, the pacing engine, lean.
"""
import sys
if "/opt/trn_rl_repo" not in sys.path:
    sys.path.insert(0, "/opt/trn_rl_repo")

import numpy as np
import ml_dtypes
import concourse.bass as bass
from concourse import bacc
import concourse.tile as tile
import concourse.mybir as mybir
from concourse.bass_utils import run_bass_kernel_spmd

AF = mybir.ActivationFunctionType
OP = mybir.AluOpType
DR = mybir.MatmulPerfMode.DoubleRow
F32, F16, F8 = mybir.dt.float32, mybir.dt.float16, mybir.dt.float8e4
E4NP = ml_dtypes.float8_e4m3  # TRN float8e4 (bias 7, max 240)

N_CORES = 8
B_FULL, I_FEAT, O_FEAT = 8192, 1024, 1024
B_LOC = B_FULL // N_CORES
BH = B_LOC // 2
N_CHUNK = I_FEAT // 128

MU = 1.0 / 12.0
S = 2.0 ** 18
ALPHA_B = -384.0
BETA_B = S / ALPHA_B            # -682.666…
CENTERS = (-0.6, -0.2, 0.2, 0.6)

_COMPILED = None


def _build_program():
    nc = bacc.Bacc("TRN2", target_bir_lowering=False, debug=False)
    xT = nc.dram_tensor("xT", [I_FEAT, B_LOC], F16, kind="ExternalInput").ap()
    wp = nc.dram_tensor("wp", [N_CHUNK, 128, 4, O_FEAT], F16,
                        kind="ExternalInput").ap()
    wb = nc.dram_tensor("wb", [N_CHUNK, 128, 4, O_FEAT], F8,
                        kind="ExternalInput").ap()
    out = nc.dram_tensor("out", [B_LOC, O_FEAT], F16, kind="ExternalOutput").ap()

    dve, act, gps = nc.vector, nc.scalar, nc.gpsimd

    # activation() resolves float bias via the const-AP registry; register
    # the Abs biases (-10*center) this kernel uses. No barrier needed: the
    # first consumer sits on the ACT queue behind ops that wait on DMAs.
    def reg_const(v):
        key = (F32, float(v))
        if key not in nc.const_aps.aps:
            t = nc.alloc_sbuf_tensor(f"constk-{len(nc.const_aps.aps)}", [128, 1], F32)
            nc.gpsimd.memset(t.ap(), float(v))
            nc.const_aps.aps[key] = t.ap()
    for c in CENTERS:
        reg_const(-10.0 * c)

    with tile.TileContext(nc) as tc:
        with tc.tile_pool(name="xin", bufs=2) as xpool, \
             tc.tile_pool(name="mid", bufs=2) as mid, \
             tc.tile_pool(name="feat", bufs=2) as fpool, \
             tc.tile_pool(name="wres", bufs=1) as wres, \
             tc.tile_pool(name="warm", bufs=1) as wpool, \
             tc.tile_pool(name="outsb", bufs=4) as opool, \
             tc.tile_pool(name="psum", bufs=1, space="PSUM") as pspool:

            # Resident weights: 8x fp16 poly (1MB) + 8x fp8 bump (0.5MB).
            # DMA ordering: batch-half-0 chunk-0 x FIRST (the feature chain
            # needs it within ~2us), then chunk-0 weights; later chunks'
            # weights prefetched from inside the chunk loop, staying ahead
            # of the matmul stream without blocking the x DMAs.
            wpoly_sb = [None] * N_CHUNK
            wbump_sb = [None] * N_CHUNK
            for f in range(N_CHUNK):
                wpoly_sb[f] = wres.tile([128, 4, O_FEAT], F16,
                                        tag=f"wp{f}", name=f"wp{f}")
                wbump_sb[f] = wres.tile([128, 4, O_FEAT], F8,
                                        tag=f"wb{f}", name=f"wb{f}")
            # x first on the queue; weight DMAs follow (measured faster
            # than triggering weights from the ACT or GPSIMD queues, whose
            # compute ops delay the triggers).
            xin0 = xpool.tile([128, BH], F16, tag="x", name="x")
            nc.sync.dma_start(xin0[:], xT[0:128, 0:BH])
            nc.sync.dma_start(wpoly_sb[0][:], wp[0, :, :, :])
            nc.sync.dma_start(wbump_sb[0][:], wb[0, :, :, :])

            # HAM warmup: dummy matmuls so the PE clock-gate reaches 8/8
            # before the real stream begins.
            warm16 = wpool.tile([128, 512], F16, tag="wrm", name="wrm")
            nc.gpsimd.memset(warm16[:], 0.0)
            warm_ps = pspool.tile([128, 512], F32, tag="ps0", name="ps0w")
            for _ in range(12):
                nc.tensor.matmul(warm_ps[:], warm16[:, 0:128], warm16[:],
                                 start=True, stop=True)

            for bh in range(2):
                if bh == 1:
                    # bridge the inter-half PE gap so the HAM clock-gate
                    # stays at 8/8 through the batch-half transition
                    for _ in range(12):
                        nc.tensor.matmul(warm_ps[:], warm16[:, 0:128],
                                         warm16[:], start=True, stop=True)
                psums = [pspool.tile([128, 512], F32, tag=f"ps{j}", name=f"ps{j}")
                         for j in range(8)]   # j = bt*2 + oh
                tail_feats = None
                for f in range(N_CHUNK):
                    if bh == 0 and f == 0:
                        xin = xin0
                    else:
                        xin = xpool.tile([128, BH], F16, tag="x", name="x")
                        nc.sync.dma_start(
                            xin[:],
                            xT[f * 128:(f + 1) * 128, bh * BH:(bh + 1) * BH])
                    if bh == 0 and f + 1 < N_CHUNK:
                        nc.sync.dma_start(wpoly_sb[f + 1][:], wp[f + 1, :, :, :])
                        nc.sync.dma_start(wbump_sb[f + 1][:], wb[f + 1, :, :, :])

                    # fp16 rows: silu, z, z^2, z^3 (alpha=1; scales live in
                    # the fp16 weights)
                    sl = fpool.tile([128, BH], F16, tag="silu", name="silu")
                    act.activation(sl[:], xin[:], AF.Silu)
                    zc = fpool.tile([128, BH], F16, tag="zc", name="zc")
                    dve.tensor_scalar(zc[:], xin[:], -1.0, 1.0, OP.max, OP.min)
                    last = (bh == 1 and f == N_CHUNK - 1)
                    z2 = fpool.tile([128, BH], F16, tag="z2", name="z2")
                    (dve if last else gps).tensor_tensor(z2[:], zc[:], zc[:], OP.mult)
                    z3 = fpool.tile([128, BH], F16, tag="z3", name="z3")
                    dve.tensor_tensor(z3[:], z2[:], zc[:], OP.mult)
                    prows = (sl, zc, z2, z3)

                    # fp8 bump rows -> one tile [128, 4, BH].
                    # u' = 10|zc-c| folds all cube scaling: an' = 10*an,
                    # an'^3 = 1000*an^3, so plain products give the scaled
                    # cubes. fb = an'^3 - (4*bn'^3 - 32) = -384(B - 1/12).
                    # All four bumps are processed as single batched
                    # [128, 4*BH] ops per wave (4x fewer instructions,
                    # amortized fixed overheads and semaphores).
                    fb = fpool.tile([128, 4, BH], F8, tag="fb", name="fb")
                    ua = mid.tile([128, 4, BH], F16, tag="ua", name="ua")
                    for j, c in enumerate(CENTERS):
                        act.activation(ua[:, j, :], zc[:], AF.Abs, scale=10.0,
                                       bias=float(-10.0 * c))
                    ana = mid.tile([128, 4, BH], F16, tag="ana", name="ana")
                    dve.tensor_scalar(ana[:], ua[:], 8.0, 0.0,
                                      OP.subtract, OP.min)
                    bna = mid.tile([128, 4, BH], F16, tag="bna", name="bna")
                    dve.tensor_scalar(bna[:], ana[:], 4.0, 0.0,
                                      OP.add, OP.min)
                    qaa = mid.tile([128, 4, BH], F16, tag="qaa", name="qaa")
                    act.activation(qaa[:], ana[:], AF.Square)
                    qba = mid.tile([128, 4, BH], F16, tag="qba", name="qba")
                    dve.tensor_tensor(qba[:], bna[:], bna[:], OP.mult)
                    caa = mid.tile([128, 4, BH], F16, tag="caa", name="caa")
                    dve.tensor_tensor(caa[:], qaa[:], ana[:], OP.mult)
                    cba = mid.tile([128, 4, BH], F16, tag="cba", name="cba")
                    (dve if last else gps).tensor_tensor(cba[:], qba[:], bna[:], OP.mult)
                    cbsa = mid.tile([128, 4, BH], F16, tag="cbsa", name="cbsa")
                    act.activation(cbsa[:], cba[:], AF.Copy,
                                   scale=4.0, bias=-32.0)
                    dve.tensor_tensor(fb[:, 0:2, :], caa[:, 0:2, :],
                                      cbsa[:, 0:2, :], OP.subtract)
                    dve.tensor_tensor(fb[:, 2:4, :], caa[:, 2:4, :],
                                      cbsa[:, 2:4, :], OP.subtract)

                    # matmuls: 6 slots per chunk (4 fp16 + 2 DoubleRow),
                    # features stationary, one lhsT feeds both oh halves.
                    # Final chunk handled bank-major below for drain overlap.
                    if f == N_CHUNK - 1:
                        tail_feats = (prows, fb)
                        continue
                    for bt in range(4):
                        for r in range(4):
                            lhsT = prows[r][:, bt * 128:(bt + 1) * 128]
                            for oh in range(2):
                                nc.tensor.matmul(
                                    psums[bt * 2 + oh][:], lhsT,
                                    wpoly_sb[f][:, r, oh * 512:(oh + 1) * 512],
                                    start=(f == 0 and r == 0), stop=False)
                        for p in range(2):
                            lhsT = fb[:, 2 * p:2 * p + 2, bt * 128:(bt + 1) * 128]
                            for oh in range(2):
                                nc.tensor.matmul(
                                    psums[bt * 2 + oh][:], lhsT,
                                    wb_pair(wbump_sb[f], p, oh),
                                    start=False, stop=False, perf_mode=DR)

                # bank-major tail over the last chunk: bank j finishes its 6
                # contributions then drains while the PE works on later banks.
                prows, fb = tail_feats
                fl = N_CHUNK - 1
                for j in range(8):
                    bt, oh = j // 2, j % 2
                    for r in range(4):
                        nc.tensor.matmul(
                            psums[j][:],
                            prows[r][:, bt * 128:(bt + 1) * 128],
                            wpoly_sb[fl][:, r, oh * 512:(oh + 1) * 512],
                            start=False, stop=False)
                    for p in range(2):
                        nc.tensor.matmul(
                            psums[j][:],
                            fb[:, 2 * p:2 * p + 2, bt * 128:(bt + 1) * 128],
                            wb_pair(wbump_sb[fl], p, oh),
                            start=False, stop=(p == 1), perf_mode=DR)
                    rows = slice(bh * BH + bt * 128, bh * BH + (bt + 1) * 128)
                    # banks (bt,0) and (bt,1) drain into one [128,1024]
                    # tile; a single DMA ships both, halving the tail's
                    # serialized trigger chain. Mid-kernel drains stay off
                    # DVE (the pacing engine); in the final tail DVE is
                    # idle, so alternate engines there.
                    if oh == 0:
                        ob = opool.tile([128, 1024], F16, tag=f"obt{bt % 2}",
                                        name=f"obt{bt % 2}")
                        ob_hold = ob
                    else:
                        ob = ob_hold
                    dst = ob[:, oh * 512:(oh + 1) * 512]
                    if bh == 1 and oh == 0:
                        dve.tensor_scalar_mul(dst, psums[j][:], 1.0 / S)
                    else:
                        act.activation(dst, psums[j][:], AF.Copy,
                                       scale=1.0 / S)
                    if oh == 1:
                        nc.sync.dma_start(out[rows, :], ob[:])
    nc.compile()
    return nc


def wb_pair(wtile, p, oh):
    """rhs AP [128, 2, 512] for DoubleRow pair p, output half oh."""
    return wtile[:, 2 * p:2 * p + 2, oh * 512:(oh + 1) * 512]


def _get_program():
    global _COMPILED
    if _COMPILED is None:
        _COMPILED = _build_program()
    return _COMPILED


# ---------------- host-side weight preparation ----------------

_GRID = np.linspace(-2.2, 2.2, 12)
_H_KNOT = 0.4


def _bspline_basis_np(z):
    xg = z[..., None]
    basis = ((xg >= _GRID[:-1]) & (xg < _GRID[1:])).astype(np.float64)
    for k in range(1, 4):
        ld = _GRID[k:-1] - _GRID[:-(k + 1)]; ld = np.where(ld == 0, 1, ld)
        rd = _GRID[k + 1:] - _GRID[1:-k]; rd = np.where(rd == 0, 1, rd)
        basis = ((xg - _GRID[:-(k + 1)]) / ld * basis[..., :-1]
                 + (_GRID[k + 1:] - xg) / rd * basis[..., 1:])
    return basis


def _bump_np(z, c):
    u = np.abs(z - c) / _H_KNOT
    return (np.maximum(2 - u, 0) ** 3 - 4 * np.maximum(1 - u, 0) ** 3) / 6


def _features_np(z):
    return np.stack([np.ones_like(z), z, z * z, z ** 3,
                     _bump_np(z, -0.6), _bump_np(z, -0.2),
                     _bump_np(z, 0.2), _bump_np(z, 0.6)], axis=-1)


def _fold_matrix():
    zs = np.linspace(-1, 1, 20001)
    M, *_ = np.linalg.lstsq(_features_np(zs), _bspline_basis_np(zs), rcond=None)
    return M   # (8 features, 8 basis)


_M_FOLD = None


def _q8(v):
    return np.clip(v, -224, 224).astype(E4NP).astype(np.float64)


def _prep_weights(x, base_weight, spline_weight):
    global _M_FOLD
    if _M_FOLD is None:
        _M_FOLD = _fold_matrix()
    bw = np.ascontiguousarray(base_weight, dtype=np.float64)
    sw = np.ascontiguousarray(spline_weight, dtype=np.float64)
    O, I = bw.shape

    W = np.einsum('oig,fg->oif', sw, _M_FOLD)       # (O, I, 8)
    W[:, :, 0] += MU * W[:, :, 4:8].sum(axis=2)     # centering -> bias col

    # feature stats on an x subsample (centered features, device scaling)
    rng = np.random.default_rng(12345)
    zs = np.clip(x[rng.choice(x.shape[0], 512, replace=False)].ravel(), -1, 1)
    Fs = _features_np(zs)
    Fs[:, 4:8] -= MU
    H = Fs.T @ Fs / len(zs)

    # feature-side absorption: project deterministic e4m3 feature error of
    # each bump row onto the feature span; pre-subtract from weights.
    dF = np.empty_like(Fs[:, 4:8])
    for j in range(4):
        v = Fs[:, 4 + j] * ALPHA_B
        dF[:, j] = _q8(v) / ALPHA_B - Fs[:, 4 + j]
    Hreg = H + 1e-10 * np.eye(8) * H.max()
    C = np.linalg.solve(Hreg, Fs.T @ dF / len(zs))   # (8, 4) projection coeffs
    W -= np.einsum('oig,fg->oif', W[:, :, 4:8], C)

    # GPTQ: quantize bump rows (e4m3 at BETA_B) with OBS compensation into
    # all not-yet-quantized coords (incl. exact poly + bias rows).
    Wq = W.reshape(-1, 8)
    Hi = np.linalg.inv(Hreg)
    done = []
    for g in (4, 5, 6, 7):
        qg = _q8(Wq[:, g] * BETA_B) / BETA_B
        e = (Wq[:, g] - qg) / Hi[g, g]
        Wq[:, g] = qg
        done.append(g)
        rem = [j for j in range(8) if j not in done]
        Wq[:, rem] -= np.outer(e, Hi[g, rem])
        Hi = Hi - np.outer(Hi[:, g], Hi[g, :]) / Hi[g, g]
        Hi[g, :] = 0; Hi[:, g] = 0; Hi[g, g] = 1.0
    W = Wq.reshape(O, I, 8)

    bias = W[:, :, 0].sum(axis=1)                   # (O,)

    # device arrays
    wp = np.empty((N_CHUNK, 128, 4, O_FEAT), dtype=np.float16)
    wb8 = np.empty((N_CHUNK, 128, 4, O_FEAT), dtype=E4NP)
    for f in range(N_CHUNK):
        rows = slice(f * 128, (f + 1) * 128)
        wp[f, :, 0, :] = (bw.T[rows, :] * S).astype(np.float16)
        for r in (1, 2, 3):
            wp[f, :, r, :] = (W[:, rows, r].T * S).astype(np.float16)
        for j in range(4):
            wb8[f, :, j, :] = np.clip(W[:, rows, 4 + j].T * BETA_B,
                                      -224, 224).astype(E4NP)
    return wp, wb8, bias.astype(np.float32)


def _run(x, base_weight, spline_weight, trace=False, tmpdir=None):
    nc = _get_program()
    x64 = np.ascontiguousarray(x, dtype=np.float64)
    x16 = x64.astype(np.float16)
    wp, wb8, bias = _prep_weights(x64, base_weight, spline_weight)
    in_maps = []
    for c in range(N_CORES):
        xc = np.ascontiguousarray(x16[c * B_LOC:(c + 1) * B_LOC, :].T)
        in_maps.append({"xT": xc, "wp": wp, "wb": wb8})
    res = run_bass_kernel_spmd(nc, in_maps, core_ids=list(range(N_CORES)),
                               trace=trace, tmpdir=tmpdir)
    full = np.concatenate([res.results[c]["out"] for c in range(N_CORES)],
                          axis=0).astype(np.float32)
    full += bias[None, :]
    return full, res


def kernel(x, base_weight, spline_weight):
    out, _ = _run(x, base_weight, spline_weight, trace=False)
    return out
